# revision 21
# baseline (speedup 1.0000x reference)
"""Cross-attention (image<->text) kernel for TRN2, 8-core SPMD.

Problem: nn_CrossAttention. B=4, C=256, H=W=64 (Ni=4096), Lt=4096, Hd=128.

Sharding: 8 cores = 4 batches x 2 query-token halves. Each core computes
  att1 (img->text): queries = image tokens [half of 4096], keys/values = all text tokens
  att2 (text->img): queries = text tokens [half of 4096], keys/values = all image tokens
Outputs are disjoint slices -> no collectives; host gathers + transposes.

Per-core algorithm (fp32 PSUM accumulation throughout):
  - Host folds BN (eval) + the sqrt(Hd) score scale into the projection
    weights, pre-transposes them, and converts x to fp16. V^T tiles (plus a
    ones column that yields the softmax denominator inside the AV
    accumulation) come host-pretransposed in bf16.
  - Projections (fp16) evict PSUM->SBUF via ACT twice: once fp16 (exact
    q/k) and once fp8e4m3 (stats copy); two SWDGE DMAs (Pool queue, off the
    shared HWDGE) lay the fp8 copy out as [64, 2, N] k-tile pairs for
    DoubleRow matmuls.
  - Pass 1 (stats) runs entirely in fp8 DoubleRow (0.5 PE cycles/row):
    S8 = Q8 K8^T in [q, m] tiles; per-row max via DVE negated max-reduce.
    The resulting per-query shift b is only a ~+-30-accurate estimate of
    the true row max -- safe because P~ is bf16 (exp range e^+-88) and any
    consistent per-row shift cancels exactly in the final normalization.
  - Pass 2: exact fp16 scores S^T in [m, q] layout; the per-query -b shift
    is added with a single fp8 DoubleRow rank-1 matmul: lhsT = a row of
    2.0s, rhs = the -b/4 row broadcast (stride-0) over both k-tiles, so
    2*(-b/4)*2 = -b. (/4 keeps magnitudes < ~170: the device float8e4 is
    IEEE e4m3, max finite 240 then inf.) Half the PE cost of an fp16
    rank-1 pair. exp on ACT -> bf16 P~; AV: out[q, C+1] += P~^T.T @ [V|1];
    normalization is one batched reciprocal + per-row scale at eviction.
  - Scheduling: DMA priority order (tiny weights first, 512-token leading
    x chunks) so the first projection starts ~2us in; stats pairs are
    emitted as soon as their q8/k8 chunks exist (p-major); the AV mc loop
    is software-pipelined (next chunk's score matmuls precede this chunk's
    AV) with next-quarter stats pairs + remaining projection tiles
    distributed over the mc slots; unnormalized-output staging runs on the
    otherwise-idle Pool engine so the next quarter's first AV matmul isn't
    WAR-blocked behind ACT's exp queue.
"""

import numpy as np
import ml_dtypes

import concourse.bacc as bacc
import concourse.tile as tile
from concourse import mybir
from concourse.bass_utils import run_bass_kernel_spmd

F32 = mybir.dt.float32
F16 = mybir.dt.float16
BF16 = mybir.dt.bfloat16
F8 = mybir.dt.float8e4
AF = mybir.ActivationFunctionType
ALU = mybir.AluOpType
AX = mybir.AxisListType
DRM = mybir.MatmulPerfMode.DoubleRow

B, C, HD = 4, 256, 128
NI, LT = 4096, 4096
NQ = 2048        # query tokens per core (half)
M = 4096         # kv tokens (full)
SQ = float(128.0 ** 0.25)   # sqrt(sqrt(Hd)) folded into each of q and k
BN_EPS = 1e-5

N_CORES = 8


def _proj_tile(nc, pools, out_t, f8, wT, bias_ap, x, n0):
    """One 512-token projection tile: 2 matmuls (c halves) + fp16 ACT
    eviction + fp8 ACT eviction into tmp8, + 2 SWDGE DMAs laying the fp8
    copy out as [64, 2, 512] k-tile pairs (hd = t*64 + p) for DoubleRow.
    Uses the ps_p2 ring (keeps the stats pairs' ps_p1 ring free)."""
    ps = pools["ps_p2"].tile([128, 512], F32, tag="ps_p2", name="ps_prj")
    for ch in range(2):
        nc.tensor.matmul(ps, wT[:, ch, :], x[:, ch, n0:n0 + 512],
                         start=(ch == 0), stop=(ch == 1))
    nc.scalar.activation(out_t[:, n0:n0 + 512], ps, AF.Identity,
                         bias=bias_ap, scale=1.0)
    tmp8, qk8 = f8
    nc.scalar.activation(tmp8[:, n0:n0 + 512], ps, AF.Identity,
                         bias=bias_ap, scale=1.0)
    nc.gpsimd.dma_start(out=qk8[:, 0, n0:n0 + 512],
                        in_=tmp8[0:64, n0:n0 + 512])
    nc.gpsimd.dma_start(out=qk8[:, 1, n0:n0 + 512],
                        in_=tmp8[64:128, n0:n0 + 512])


def _stats_pair(nc, pools, st, qt, p):
    """One pass-1 unit: 2 fp8 DoubleRow score matmuls into one 2-bank psum
    tile + ONE negated max-reduce [128,2,512]->[128,2] on DVE."""
    q8, k8 = st["q8"], st["k8"]
    q_sl = q8[:, :, qt * 128:(qt + 1) * 128]
    ps = pools["ps_p1"].tile([128, 1024], F32, tag="ps_p1", name="ps_s",
                             bufs=1)
    for j in range(2):
        w = 2 * p + j
        nc.tensor.matmul(ps[:, j * 512:(j + 1) * 512], q_sl,
                         k8[:, :, w * 512:(w + 1) * 512],
                         start=True, stop=True, perf_mode=DRM)
    nc.vector.tensor_reduce(st["negm_all"][:, qt, 2 * p:2 * p + 2],
                            ps.rearrange("a (j w) -> a j w", w=512),
                            axis=AX.X, op=ALU.max, negate=True)


def _attn_stats_finalize_half(nc, pools, st, nq, h):
    """Combine pair maxes for qtile pair h of quarter nq -> fp8 -b/4 values,
    scattered into the quarter's bias row in linear-q order (2 gathers)."""
    sp, tag = pools["stats"], st["tag"]
    sl = slice(nq * 4 + 2 * h, nq * 4 + 2 * h + 2)
    neg_mt = sp.tile([128, 2], F32, name=f"negM_{tag}_{nq}_{h}",
                     tag=f"negM_{tag}", bufs=2)
    nc.vector.tensor_reduce(neg_mt, st["negm_all"][:, sl, :], axis=AX.X,
                            op=ALU.min)
    neg_b8 = sp.tile([128, 2], F8, tag=f"negb8_{tag}",
                     name=f"negb8_{tag}_{nq}_{h}", bufs=2)
    nc.vector.tensor_scalar_mul(neg_b8, neg_mt, 0.25)
    for s in range(2):
        b0 = nq * 512 + h * 256 + s * 128
        # HWDGE (sync), not SWDGE: these gathers gate the next quarter's
        # bias matmuls, and the Pool desc-gen path adds ~1us each
        nc.sync.dma_start(out=st["negb8"][0:1, b0:b0 + 128],
                          in_=neg_b8[:, s:s + 1])


def _stats_pairs(nc, pools, st, nq):
    """16 pair closures for one quarter's stats, p-major (each fresh pair of
    k8 tiles unlocks four more pairs)."""
    return [lambda qt=nq * 4 + i, p=p: _stats_pair(nc, pools, st, qt, p)
            for p in range(4) for i in range(4)]


def _stats_fins(nc, pools, st, nq):
    return [lambda h=h: _attn_stats_finalize_half(nc, pools, st, nq, h)
            for h in range(2)]


def _attn_av_quarter(nc, pools, st, nq, fillers=()):
    """Pass-2 S^T + DoubleRow rank-1 bias + exp + AV for one 512-query
    quarter.

    mc loop software-pipelined by one iteration (bias+score matmuls of chunk
    mc+1 are emitted BEFORE the AV matmuls of chunk mc); `fillers`
    (next-quarter stats + remaining projection tiles) are spread evenly
    over the 32 mc slots."""
    qT, kT, v, tag = st["qT"], st["kT"], st["v"], st["tag"]
    twos8 = pools["twos8"]
    n_mc = M // 128
    v3 = v.rearrange("p (mc w) -> p mc w", w=257)
    n0 = nq * 512
    negb8_b = st["negb8"][0:1, n0:n0 + 512].unsqueeze(1) \
        .broadcast_to((1, 2, 512))
    out_ps = [pools["ps_out"].tile([128, 257], F32, tag=f"out_ps{s}",
                                   name=f"out_ps{s}_{tag}_{nq}")
              for s in range(4)]
    st["out_ps"] = out_ps

    def scores(mc):
        ps = pools["ps_p2"].tile([128, 512], F32, tag="ps_p2", name="ps_p2c")
        # the full-width score matmul OPENS the group (it depends only on
        # qT/kT); the DoubleRow bias closes it, so a late-arriving finalize
        # DMA never blocks the score matmul itself
        nc.tensor.matmul(ps, kT[:, mc * 128:(mc + 1) * 128],
                         qT[:, n0:n0 + 512], start=True, stop=False)
        nc.tensor.matmul(ps, twos8, negb8_b, start=False, stop=True,
                         perf_mode=DRM, skip_group_check=True)
        pt = pools["pt"].tile([128, 512], BF16, tag="pt",
                              name=f"pt_{tag}_{nq}_{mc}")
        nc.scalar.activation(pt, ps, AF.Exp)
        return pt

    fillers = list(fillers)
    # stats run one quarter ahead: fillers = [fin(q+1) x2, pairs(q+2) x16].
    # The two fins go in the first two slots (their inputs were reduced last
    # quarter; the bias-row gathers then land ~30 mc before they're needed);
    # pairs are spaced 2 mc apart so a pair's matmuls never WAR-stall on the
    # single-buffer ps_p1 tile still being reduced.
    slot = {}
    for k in range(len(fillers)):
        slot.setdefault(k if k < 2 else min(2 * k - 2, n_mc - 1), []).append(k)
    pt_cur = scores(0)
    for mc in range(n_mc):
        pt_nxt = scores(mc + 1) if mc + 1 < n_mc else None
        for s in range(4):
            nc.tensor.matmul(out_ps[s], pt_cur[:, s * 128:(s + 1) * 128],
                             v3[:, mc, :],
                             start=(mc == 0), stop=(mc == n_mc - 1))
        for k in slot.get(mc, ()):
            fillers[k]()
        pt_cur = pt_nxt


def _attn_store_quarter(nc, pools, st, nq):
    """Normalize straight out of PSUM on DVE (per-s reciprocal + scale) +
    store DMA. Skipping a raw-staging copy frees each out_ps tile ~0.5us
    after its last AV matmul, so the next quarter's first AV matmul isn't
    WAR-blocked behind an eviction queue."""
    sp, tag, out_dram = pools["stats"], st["tag"], st["out_dram"]
    out_ps = st["out_ps"]
    for s in range(4):
        qt = nq * 4 + s
        recip = sp.tile([128, 1], F32, tag=f"recip_{tag}",
                        name=f"recip_{tag}_{qt}", bufs=2)
        nc.vector.reciprocal(recip, out_ps[s][:, 256:257])
        ostage = pools["ostage"].tile([128, 256], F32, tag="ostage",
                                      name=f"ostage_{tag}_{qt}")
        nc.vector.tensor_scalar_mul(ostage, out_ps[s][:, 0:256], recip)
        nc.sync.dma_start(out=out_dram[qt * 128:(qt + 1) * 128, :], in_=ostage)


def _attn_state(nc, pools, tag, qT, kT, q8, k8, v, out_dram):
    sp = pools["stats"]
    n_qt = NQ // 128
    st = {"tag": tag, "qT": qT, "kT": kT, "q8": q8, "k8": k8, "v": v,
          "out_dram": out_dram}
    st["negb8"] = sp.tile([1, NQ], F8, name=f"negb8row_{tag}",
                          tag=f"negb8row_{tag}", bufs=1)
    st["negm_all"] = sp.tile([128, n_qt, 8], F32, name=f"negmall_{tag}",
                             tag=f"negmall_{tag}", bufs=1)
    return st


def build_nc(reps=1):
    """Build the SPMD Bass program (identical for all cores).

    Query tokens are always slice [0:NQ] of the token axis. The host feeds
    half-1 cores an x whose two token halves are swapped: queries then sit in
    [0:NQ], while keys/values see a permuted-but-consistent full token set
    (softmax+AV are invariant to a joint permutation of keys and values)."""
    nc = bacc.Bacc(None)

    x_i = nc.dram_tensor("x_i", [2, 128, NI], F16, kind="ExternalInput")
    x_t = nc.dram_tensor("x_t", [2, 128, LT], F16, kind="ExternalInput")
    # host-pretransposed V (+ ones column): [m-in-chunk, mc, C+1]
    v_td = nc.dram_tensor("v_td", [128, M // 128, 257], BF16, kind="ExternalInput")
    v_id = nc.dram_tensor("v_id", [128, M // 128, 257], BF16, kind="ExternalInput")
    # host-folded, pre-transposed fp16 weights, packed into ONE tensor
    # [c-in-half, ch, which(wq,wk,wtq,wtk), hd] so the prologue needs a
    # single tiny weight DMA before the first projection can start
    wpack = nc.dram_tensor("wpack", [128, 2, 4, 128], F16, kind="ExternalInput")
    # host-folded projection biases [128, 4]: i_q, i_k, t_k, t_q
    biases = nc.dram_tensor("biases", [128, 4], F32, kind="ExternalInput")
    out_i = nc.dram_tensor("out_i", [NQ, C], F32, kind="ExternalOutput")
    out_t = nc.dram_tensor("out_t", [NQ, C], F32, kind="ExternalOutput")

    with tile.TileContext(nc) as tc:
        import contextlib
        with contextlib.ExitStack() as ctx:

            pools = {}
            pools["consts"] = ctx.enter_context(tc.tile_pool(name="consts", bufs=1))
            pools["stats"] = ctx.enter_context(tc.tile_pool(name="stats", bufs=2))
            pools["xpool"] = ctx.enter_context(tc.tile_pool(name="xpool", bufs=1))
            pools["vpool"] = ctx.enter_context(tc.tile_pool(name="vpool", bufs=1))
            pools["qkpool"] = ctx.enter_context(tc.tile_pool(name="qkpool", bufs=1))
            pools["f8pool"] = ctx.enter_context(tc.tile_pool(name="f8pool", bufs=1))
            pools["pt"] = ctx.enter_context(tc.tile_pool(name="pt", bufs=5))
            pools["ostage"] = ctx.enter_context(tc.tile_pool(name="ostage", bufs=4))
            pools["ps_p1"] = ctx.enter_context(
                tc.tile_pool(name="ps_p1", bufs=2, space="PSUM"))
            pools["ps_p2"] = ctx.enter_context(
                tc.tile_pool(name="ps_p2", bufs=2, space="PSUM"))
            pools["ps_out"] = ctx.enter_context(
                tc.tile_pool(name="ps_out", bufs=1, space="PSUM"))

            consts = pools["consts"]

            wpack_t = consts.tile([128, 2, 4, 128], F16, name="wpack_t")
            wT = {name: wpack_t[:, :, i, :]
                  for i, name in enumerate(("wq", "wk", "wtq", "wtk"))}
            bias_t = consts.tile([128, 4], F32, name="bias_t")

            twos_f = consts.tile([1, 256], F32, name="twos_row_f")
            nc.vector.memset(twos_f, 2.0)
            twos8_t = consts.tile([1, 2, 128], F8, name="twos8_row")
            nc.vector.tensor_copy(twos8_t.rearrange("o t p -> o (t p)"), twos_f)
            pools["twos8"] = twos8_t

            for _rep in range(reps):
                qk = pools["qkpool"]
                t_kT = qk.tile([128, LT], F16, name="t_kT")
                t_qT = qk.tile([128, NQ], F16, name="t_qT")
                i_qT = qk.tile([128, NQ], F16, name="i_qT")
                i_kT = qk.tile([128, NI], F16, name="i_kT")
                f8p = pools["f8pool"]
                t_k8 = f8p.tile([64, 2, LT], F8, name="t_k8")
                t_q8 = f8p.tile([64, 2, NQ], F8, name="t_q8")
                i_q8 = f8p.tile([64, 2, NQ], F8, name="i_q8")
                i_k8 = f8p.tile([64, 2, NI], F8, name="i_k8")
                tmp8 = {"t_k": f8p.tile([128, LT], F8, name="tmp8_tk"),
                        "t_q": f8p.tile([128, NQ], F8, name="tmp8_tq"),
                        "i_q": f8p.tile([128, NQ], F8, name="tmp8_iq"),
                        "i_k": f8p.tile([128, NI], F8, name="tmp8_ik")}
                v_t = pools["vpool"].tile([128, (M // 128) * 257], BF16, name="v_t")
                v_i = pools["vpool"].tile([128, (M // 128) * 257], BF16, name="v_i")
                v3_t = v_t.rearrange("p (mc w) -> p mc w", w=257)
                v3_i = v_i.rearrange("p (mc w) -> p mc w", w=257)

                xt = pools["xpool"].tile([128, 2, LT], F16, tag="x_t", name="xt")
                xi = pools["xpool"].tile([128, 2, NI], F16, tag="x_i", name="xi")

                def xdma(dst, src, ch, c0, c1):
                    nc.sync.dma_start(out=dst[:, ch, c0:c1],
                                      in_=src[ch, :, c0:c1])

                # ---- DMA priority order on the (shared) HWDGE path:
                # tiny weights first, then 512-token leading x chunks so the
                # first projection tile can start ~2us in, then the rest ----
                nc.sync.dma_start(out=wpack_t, in_=wpack[:, :, :, :])
                nc.sync.dma_start(out=bias_t, in_=biases[:, :])
                for ch in range(2):
                    xdma(xt, x_t, ch, 0, 512)
                for ch in range(2):
                    xdma(xi, x_i, ch, 0, 512)
                for ch in range(2):
                    xdma(xt, x_t, ch, 512, 2048)
                for ch in range(2):
                    xdma(xi, x_i, ch, 512, 2048)
                for ch in range(2):
                    xdma(xt, x_t, ch, 2048, 4096)
                for ch in range(2):
                    xdma(xi, x_i, ch, 2048, 4096)
                nc.sync.dma_start(out=v3_t, in_=v_td[:, :, :])
                nc.sync.dma_start(out=v3_i, in_=v_id[:, :, :])

                a1 = _attn_state(nc, pools, "a1", i_qT, t_kT, i_q8, t_k8,
                                 v_t, out_i)
                a2 = _attn_state(nc, pools, "a2", t_qT, i_kT, t_q8, i_k8,
                                 v_i, out_t)

                def ptile(dst, f8key, q8t, wname, bcol, x, n0):
                    return lambda: _proj_tile(nc, pools, dst,
                                              (tmp8[f8key], q8t), wT[wname],
                                              bias_t[:, bcol:bcol + 1], x, n0)

                A = [ptile(t_kT, "t_k", t_k8, "wtk", 2, xt, n0)
                     for n0 in range(0, LT, 512)]
                Bq = [ptile(i_qT, "i_q", i_q8, "wq", 0, xi, n0)
                      for n0 in range(0, NQ, 512)]
                T = [ptile(t_qT, "t_q", t_q8, "wtq", 3, xt, n0)
                     for n0 in range(0, NQ, 512)]
                K = [ptile(i_kT, "i_k", i_k8, "wk", 1, xi, n0)
                     for n0 in range(0, NI, 512)]

                # ---- prologue: ALL projection tiles + the stats pairs of
                # quarters a1-0 AND a1-1, in dependency-local order
                # (p-major stats so each fresh pair of t_k tiles unlocks
                # four more pairs; projection tiles ordered by x-chunk
                # arrival so the in-order PE queue never blocks a ready
                # stats pair behind a waiting one) ----
                cells = [(a1, q) for q in range(NQ // 512)] + \
                        [(a2, q) for q in range(NQ // 512)]
                s0 = _stats_pairs(nc, pools, a1, 0)
                prologue = ([A[0], A[1], Bq[0], T[0]]
                            + s0[0:4] + [A[2], A[3], T[1], Bq[1], K[0]]
                            + s0[4:8] + [A[4], A[5], T[2], T[3], Bq[2], K[1]]
                            + s0[8:12] + [A[6], A[7], Bq[3], K[2], K[3]]
                            + s0[12:16]
                            + _stats_fins(nc, pools, a1, 0)
                            + [K[4], K[5], K[6], K[7]]
                            + _stats_pairs(nc, pools, a1, 1))
                for u in prologue:
                    u()

                # ---- steady state runs stats one quarter ahead: while cell
                # i computes, emit cell i+1's finalizes (pairs reduced last
                # cell) and cell i+2's pairs ----
                for i, (att, q) in enumerate(cells):
                    fillers = []
                    if i + 1 < len(cells):
                        fillers += _stats_fins(nc, pools, *cells[i + 1])
                    if i + 2 < len(cells):
                        fillers += _stats_pairs(nc, pools, *cells[i + 2])
                    _attn_av_quarter(nc, pools, att, q, fillers)
                    _attn_store_quarter(nc, pools, att, q)

    nc.compile()
    return nc


_NC_CACHE = {}


def _get_nc():
    if "nc" not in _NC_CACHE:
        _NC_CACHE["nc"] = build_nc()
    return _NC_CACHE["nc"]


def _swapped(x_flat, h):
    """x_flat [C, Ntok] fp16, token halves swapped when h==1."""
    x_flat = x_flat.astype(np.float16)
    if h:
        n = x_flat.shape[1]
        x_flat = np.concatenate([x_flat[:, n // 2:], x_flat[:, :n // 2]], axis=1)
    return x_flat


def _v_pack(xs):
    """xs [C, M] -> [128, M/128, 257] x^T chunks + ones column (bf16)."""
    v = np.empty((128, M // 128, 257), ml_dtypes.bfloat16)
    v[:, :, 0:256] = xs.astype(np.float32).T.reshape(
        M // 128, 128, C).transpose(1, 0, 2).astype(ml_dtypes.bfloat16)
    v[:, :, 256] = 1.0
    return np.ascontiguousarray(v)


def _fold_weight(w, scale):
    """w [Hd, C] * per-row scale -> pre-transposed fp16 [c-in-half, ch, hd]."""
    wp = (w * scale[:, None]).astype(np.float32)
    arr = wp.T.reshape(2, 128, 128).transpose(1, 0, 2)
    return np.ascontiguousarray(arr.astype(np.float16))


def run_spmd(inputs, **kw):
    """Build in_maps, run on 8 cores, return BassKernelResults."""
    nc = _get_nc()
    s_q = inputs["bnq_gamma"] / np.sqrt(inputs["bnq_var"] + BN_EPS)
    s_k = inputs["bnk_gamma"] / np.sqrt(inputs["bnk_var"] + BN_EPS)
    wq = _fold_weight(inputs["w_img_q"], s_q * SQ)
    wk = _fold_weight(inputs["w_img_k"], s_k * SQ)
    wtq = _fold_weight(inputs["w_text_q"], np.full(HD, SQ, np.float32))
    wtk = _fold_weight(inputs["w_text_k"], np.full(HD, SQ, np.float32))
    wpack = np.ascontiguousarray(np.stack([wq, wk, wtq, wtk], axis=2))
    biases = np.ascontiguousarray(np.stack([
        (inputs["bnq_beta"] - inputs["bnq_mean"] * s_q) * SQ,
        (inputs["bnk_beta"] - inputs["bnk_mean"] * s_k) * SQ,
        inputs["b_text_k"] * SQ,
        inputs["b_text_q"] * SQ,
    ], axis=1).astype(np.float32))
    in_maps = []
    for core in range(N_CORES):
        b, h = core // 2, core % 2
        xsi = _swapped(inputs["input_i"][b].reshape(C, NI), h)
        xst = _swapped(inputs["input_t"][b].reshape(C, LT), h)
        m = {
            "x_i": np.ascontiguousarray(xsi.reshape(2, 128, NI)),
            "x_t": np.ascontiguousarray(xst.reshape(2, 128, LT)),
            "v_td": _v_pack(xst), "v_id": _v_pack(xsi),
            "wpack": wpack, "biases": biases,
        }
        in_maps.append(m)
    res = run_bass_kernel_spmd(nc, in_maps, list(range(N_CORES)), **kw)
    return res


def gather(res):
    output_i = np.empty((B, NI, C), np.float32)
    output_t = np.empty((B, LT, C), np.float32)
    for core in range(N_CORES):
        b, h = core // 2, core % 2
        r = res.results[core]
        output_i[b, h * NQ:(h + 1) * NQ, :] = np.asarray(r["out_i"])
        output_t[b, h * NQ:(h + 1) * NQ, :] = np.asarray(r["out_t"])
    return (output_i, output_t)


def kernel(**inputs):
    inputs = {k: np.asarray(v, dtype=np.float32) for k, v in inputs.items()}
    res = run_spmd(inputs)
    return gather(res)


# revision 28
# speedup vs baseline: 1.0330x; 1.0330x over previous
"""Cross-attention (image<->text) kernel for TRN2, 8-core SPMD.

Problem: nn_CrossAttention. B=4, C=256, H=W=64 (Ni=4096), Lt=4096, Hd=128.

Sharding: 8 cores = 4 batches x 2 query-token halves. Each core computes
  att1 (img->text): queries = image tokens [half of 4096], keys/values = all text tokens
  att2 (text->img): queries = text tokens [half of 4096], keys/values = all image tokens
Outputs are disjoint slices -> no collectives; host gathers + transposes.

Per-core algorithm (fp32 PSUM accumulation throughout):
  - Host folds BN (eval) + the sqrt(Hd) score scale into the projection
    weights, pre-transposes them, and converts x to fp16. V^T tiles (plus a
    ones column that yields the softmax denominator inside the AV
    accumulation) come host-pretransposed in bf16.
  - Projections (fp16) evict PSUM->SBUF via ACT twice: once fp16 (exact
    q/k) and once fp8e4m3 (stats copy); two SWDGE DMAs (Pool queue, off the
    shared HWDGE) lay the fp8 copy out as [64, 2, N] k-tile pairs for
    DoubleRow matmuls.
  - Pass 1 (stats) runs entirely in fp8 DoubleRow (0.5 PE cycles/row):
    S8 = Q8 K8^T in [q, m] tiles; per-row max via DVE negated max-reduce.
    The resulting per-query shift b is only a ~+-30-accurate estimate of
    the true row max -- safe because P~ is bf16 (exp range e^+-88) and any
    consistent per-row shift cancels exactly in the final normalization.
  - Pass 2: exact fp16 scores S^T in [m, q] layout; the per-query -b shift
    is added with a single fp8 DoubleRow rank-1 matmul: lhsT = a row of
    2.0s, rhs = the -b/4 row broadcast (stride-0) over both k-tiles, so
    2*(-b/4)*2 = -b. (/4 keeps magnitudes < ~170: the device float8e4 is
    IEEE e4m3, max finite 240 then inf.) Half the PE cost of an fp16
    rank-1 pair. exp on ACT -> bf16 P~; AV: out[q, C+1] += P~^T.T @ [V|1];
    normalization is one batched reciprocal + per-row scale at eviction.
  - Scheduling: DMA priority order (tiny weights first, 512-token leading
    x chunks) so the first projection starts ~2us in; stats pairs are
    emitted as soon as their q8/k8 chunks exist (p-major); the AV mc loop
    is software-pipelined (next chunk's score matmuls precede this chunk's
    AV) with next-quarter stats pairs + remaining projection tiles
    distributed over the mc slots; unnormalized-output staging runs on the
    otherwise-idle Pool engine so the next quarter's first AV matmul isn't
    WAR-blocked behind ACT's exp queue.
"""

import numpy as np
import ml_dtypes

import concourse.bacc as bacc
import concourse.tile as tile
from concourse import mybir
from concourse.bass_utils import run_bass_kernel_spmd

F32 = mybir.dt.float32
F16 = mybir.dt.float16
BF16 = mybir.dt.bfloat16
F8 = mybir.dt.float8e4
AF = mybir.ActivationFunctionType
ALU = mybir.AluOpType
AX = mybir.AxisListType
DRM = mybir.MatmulPerfMode.DoubleRow

B, C, HD = 4, 256, 128
NI, LT = 4096, 4096
NQ = 2048        # query tokens per core (half)
M = 4096         # kv tokens (full)
SQ = float(128.0 ** 0.25)   # sqrt(sqrt(Hd)) folded into each of q and k
BN_EPS = 1e-5

N_CORES = 8


def _proj_tile(nc, pools, out_t, f8, wT, bias_ap, x, n0):
    """One 512-token projection tile: 2 matmuls (c halves) + fp16 ACT
    eviction + fp8 ACT eviction into tmp8, + 2 SWDGE DMAs laying the fp8
    copy out as [64, 2, 512] k-tile pairs (hd = t*64 + p) for DoubleRow.
    Uses the ps_p2 ring (keeps the stats pairs' ps_p1 ring free)."""
    ps = pools["ps_p2"].tile([128, 512], F32, tag="ps_p2", name="ps_prj")
    for ch in range(2):
        nc.tensor.matmul(ps, wT[:, ch, :], x[:, ch, n0:n0 + 512],
                         start=(ch == 0), stop=(ch == 1))
    nc.scalar.activation(out_t[:, n0:n0 + 512], ps, AF.Identity,
                         bias=bias_ap, scale=1.0)
    tmp8, qk8 = f8
    nc.scalar.activation(tmp8[:, n0:n0 + 512], ps, AF.Identity,
                         bias=bias_ap, scale=1.0)
    # HWDGE (sync), not SWDGE: the Pool desc-gen path serializes at ~1us
    # per DMA and a 48-DMA prologue backlog there starves the stats pairs
    nc.sync.dma_start(out=qk8[:, 0, n0:n0 + 512], in_=tmp8[0:64, n0:n0 + 512])
    nc.sync.dma_start(out=qk8[:, 1, n0:n0 + 512],
                      in_=tmp8[64:128, n0:n0 + 512])


def _stats_pair(nc, pools, st, qt, p):
    """One pass-1 unit: 2 fp8 DoubleRow score matmuls into one 2-bank psum
    tile + ONE negated max-reduce [128,2,512]->[128,2] on DVE."""
    q8, k8 = st["q8"], st["k8"]
    q_sl = q8[:, :, qt * 128:(qt + 1) * 128]
    ps = pools["ps_p1"].tile([128, 1024], F32, tag="ps_p1", name="ps_s",
                             bufs=1)
    for j in range(2):
        w = 2 * p + j
        nc.tensor.matmul(ps[:, j * 512:(j + 1) * 512], q_sl,
                         k8[:, :, w * 512:(w + 1) * 512],
                         start=True, stop=True, perf_mode=DRM)
    nc.vector.tensor_reduce(st["negm_all"][:, qt, 2 * p:2 * p + 2],
                            ps.rearrange("a (j w) -> a j w", w=512),
                            axis=AX.X, op=ALU.max, negate=True)


def _attn_stats_finalize_half(nc, pools, st, nq, h):
    """Combine pair maxes for qtile pair h of quarter nq -> fp8 -b/4 values,
    scattered into the quarter's bias row in linear-q order (2 gathers)."""
    sp, tag = pools["stats"], st["tag"]
    sl = slice(nq * 4 + 2 * h, nq * 4 + 2 * h + 2)
    neg_mt = sp.tile([128, 2], F32, name=f"negM_{tag}_{nq}_{h}",
                     tag=f"negM_{tag}", bufs=2)
    nc.vector.tensor_reduce(neg_mt, st["negm_all"][:, sl, :], axis=AX.X,
                            op=ALU.min)
    neg_b8 = sp.tile([128, 2], F8, tag=f"negb8_{tag}",
                     name=f"negb8_{tag}_{nq}_{h}", bufs=2)
    nc.vector.tensor_scalar_mul(neg_b8, neg_mt, 0.25)
    # store the -b/4 values physically in BOTH k-tiles (no stride-0
    # broadcast AP on the matmul read: its writes-dependency tracking is
    # what orders the bias matmul after these gathers). HWDGE (sync), not
    # SWDGE: these gathers gate the next quarter's bias matmuls, and the
    # Pool desc-gen path adds ~1us each.
    for t in range(2):
        for s in range(2):
            b0 = nq * 512 + h * 256 + s * 128
            nc.sync.dma_start(out=st["negb8"][0:1, t, b0:b0 + 128],
                              in_=neg_b8[:, s:s + 1])


def _stats_pairs(nc, pools, st, nq):
    """16 pair closures for one quarter's stats, p-major (each fresh pair of
    k8 tiles unlocks four more pairs)."""
    return [lambda qt=nq * 4 + i, p=p: _stats_pair(nc, pools, st, qt, p)
            for p in range(4) for i in range(4)]


def _stats_fins(nc, pools, st, nq):
    return [lambda h=h: _attn_stats_finalize_half(nc, pools, st, nq, h)
            for h in range(2)]


def _attn_av_quarter(nc, pools, st, nq, fillers=()):
    """Pass-2 S^T + DoubleRow rank-1 bias + exp + AV for one 512-query
    quarter.

    mc loop software-pipelined by one iteration (bias+score matmuls of chunk
    mc+1 are emitted BEFORE the AV matmuls of chunk mc); `fillers`
    (next-quarter stats + remaining projection tiles) are spread evenly
    over the 32 mc slots."""
    qT, kT, v, tag = st["qT"], st["kT"], st["v"], st["tag"]
    twos8 = pools["twos8"]
    n_mc = M // 128
    v3 = v.rearrange("p (mc w) -> p mc w", w=257)
    n0 = nq * 512
    negb8_b = st["negb8"][0:1, :, n0:n0 + 512]
    out_ps = [pools["ps_out"].tile([128, 257], F32, tag=f"out_ps{s}",
                                   name=f"out_ps{s}_{tag}_{nq}")
              for s in range(4)]
    st["out_ps"] = out_ps

    def scores(mc):
        ps = pools["ps_p2"].tile([128, 512], F32, tag="ps_p2", name="ps_p2c")
        # the full-width score matmul OPENS the group (it depends only on
        # qT/kT); the DoubleRow bias closes it, so a late-arriving finalize
        # DMA never blocks the score matmul itself
        nc.tensor.matmul(ps, kT[:, mc * 128:(mc + 1) * 128],
                         qT[:, n0:n0 + 512], start=True, stop=False)
        nc.tensor.matmul(ps, twos8, negb8_b, start=False, stop=True,
                         perf_mode=DRM, skip_group_check=True)
        pt = pools["pt"].tile([128, 512], BF16, tag="pt",
                              name=f"pt_{tag}_{nq}_{mc}")
        nc.scalar.activation(pt, ps, AF.Exp)
        return pt

    fillers = list(fillers)
    # stats run one quarter ahead: fillers = [fin(q+1) x2, pairs(q+2) x16].
    # The two fins go in the first two slots (their inputs were reduced last
    # quarter; the bias-row gathers then land ~30 mc before they're needed);
    # pairs are spaced 2 mc apart so a pair's matmuls never WAR-stall on the
    # single-buffer ps_p1 tile still being reduced.
    slot = {}
    for k in range(len(fillers)):
        slot.setdefault(k if k < 2 else min(2 * k - 2, n_mc - 1), []).append(k)
    pt_cur = scores(0)
    for mc in range(n_mc):
        pt_nxt = scores(mc + 1) if mc + 1 < n_mc else None
        for s in range(4):
            nc.tensor.matmul(out_ps[s], pt_cur[:, s * 128:(s + 1) * 128],
                             v3[:, mc, :],
                             start=(mc == 0), stop=(mc == n_mc - 1))
        for k in slot.get(mc, ()):
            fillers[k]()
        pt_cur = pt_nxt


def _attn_store_quarter(nc, pools, st, nq):
    """Normalize straight out of PSUM on DVE (per-s reciprocal + scale) +
    store DMA. Skipping a raw-staging copy frees each out_ps tile ~0.5us
    after its last AV matmul, so the next quarter's first AV matmul isn't
    WAR-blocked behind an eviction queue."""
    sp, tag, out_dram = pools["stats"], st["tag"], st["out_dram"]
    out_ps = st["out_ps"]
    for s in range(4):
        qt = nq * 4 + s
        recip = sp.tile([128, 1], F32, tag=f"recip_{tag}",
                        name=f"recip_{tag}_{qt}", bufs=2)
        nc.vector.reciprocal(recip, out_ps[s][:, 256:257])
        ostage = pools["ostage"].tile([128, 256], F32, tag="ostage",
                                      name=f"ostage_{tag}_{qt}")
        nc.vector.tensor_scalar_mul(ostage, out_ps[s][:, 0:256], recip)
        nc.sync.dma_start(out=out_dram[qt * 128:(qt + 1) * 128, :], in_=ostage)


def _attn_state(nc, pools, tag, qT, kT, q8, k8, v, out_dram):
    sp = pools["stats"]
    n_qt = NQ // 128
    st = {"tag": tag, "qT": qT, "kT": kT, "q8": q8, "k8": k8, "v": v,
          "out_dram": out_dram}
    st["negb8"] = sp.tile([1, 2, NQ], F8, name=f"negb8row_{tag}",
                          tag=f"negb8row_{tag}", bufs=1)
    st["negm_all"] = sp.tile([128, n_qt, 8], F32, name=f"negmall_{tag}",
                             tag=f"negmall_{tag}", bufs=1)
    return st


def build_nc(reps=1):
    """Build the SPMD Bass program (identical for all cores).

    Query tokens are always slice [0:NQ] of the token axis. The host feeds
    half-1 cores an x whose two token halves are swapped: queries then sit in
    [0:NQ], while keys/values see a permuted-but-consistent full token set
    (softmax+AV are invariant to a joint permutation of keys and values)."""
    nc = bacc.Bacc(None)

    x_i = nc.dram_tensor("x_i", [2, 128, NI], F16, kind="ExternalInput")
    x_t = nc.dram_tensor("x_t", [2, 128, LT], F16, kind="ExternalInput")
    # host-pretransposed V (+ ones column): [m-in-chunk, mc, C+1]
    v_td = nc.dram_tensor("v_td", [128, M // 128, 257], BF16, kind="ExternalInput")
    v_id = nc.dram_tensor("v_id", [128, M // 128, 257], BF16, kind="ExternalInput")
    # host-folded, pre-transposed fp16 weights, packed into ONE tensor
    # [c-in-half, ch, which(wq,wk,wtq,wtk), hd] so the prologue needs a
    # single tiny weight DMA before the first projection can start
    wpack = nc.dram_tensor("wpack", [128, 2, 4, 128], F16, kind="ExternalInput")
    # host-folded projection biases [128, 4]: i_q, i_k, t_k, t_q
    biases = nc.dram_tensor("biases", [128, 4], F32, kind="ExternalInput")
    out_i = nc.dram_tensor("out_i", [NQ, C], F32, kind="ExternalOutput")
    out_t = nc.dram_tensor("out_t", [NQ, C], F32, kind="ExternalOutput")

    with tile.TileContext(nc) as tc:
        import contextlib
        with contextlib.ExitStack() as ctx:

            pools = {}
            pools["consts"] = ctx.enter_context(tc.tile_pool(name="consts", bufs=1))
            pools["stats"] = ctx.enter_context(tc.tile_pool(name="stats", bufs=2))
            pools["xpool"] = ctx.enter_context(tc.tile_pool(name="xpool", bufs=1))
            pools["vpool"] = ctx.enter_context(tc.tile_pool(name="vpool", bufs=1))
            pools["qkpool"] = ctx.enter_context(tc.tile_pool(name="qkpool", bufs=1))
            pools["f8pool"] = ctx.enter_context(tc.tile_pool(name="f8pool", bufs=1))
            pools["pt"] = ctx.enter_context(tc.tile_pool(name="pt", bufs=5))
            pools["ostage"] = ctx.enter_context(tc.tile_pool(name="ostage", bufs=4))
            pools["ps_p1"] = ctx.enter_context(
                tc.tile_pool(name="ps_p1", bufs=2, space="PSUM"))
            pools["ps_p2"] = ctx.enter_context(
                tc.tile_pool(name="ps_p2", bufs=2, space="PSUM"))
            pools["ps_out"] = ctx.enter_context(
                tc.tile_pool(name="ps_out", bufs=1, space="PSUM"))

            consts = pools["consts"]

            wpack_t = consts.tile([128, 2, 4, 128], F16, name="wpack_t")
            wT = {name: wpack_t[:, :, i, :]
                  for i, name in enumerate(("wq", "wk", "wtq", "wtk"))}
            bias_t = consts.tile([128, 4], F32, name="bias_t")

            twos_f = consts.tile([1, 256], F32, name="twos_row_f")
            nc.vector.memset(twos_f, 2.0)
            twos8_t = consts.tile([1, 2, 128], F8, name="twos8_row")
            nc.vector.tensor_copy(twos8_t.rearrange("o t p -> o (t p)"), twos_f)
            pools["twos8"] = twos8_t

            for _rep in range(reps):
                qk = pools["qkpool"]
                t_kT = qk.tile([128, LT], F16, name="t_kT")
                t_qT = qk.tile([128, NQ], F16, name="t_qT")
                i_qT = qk.tile([128, NQ], F16, name="i_qT")
                i_kT = qk.tile([128, NI], F16, name="i_kT")
                f8p = pools["f8pool"]
                t_k8 = f8p.tile([64, 2, LT], F8, name="t_k8")
                t_q8 = f8p.tile([64, 2, NQ], F8, name="t_q8")
                i_q8 = f8p.tile([64, 2, NQ], F8, name="i_q8")
                i_k8 = f8p.tile([64, 2, NI], F8, name="i_k8")
                tmp8 = {"t_k": f8p.tile([128, LT], F8, name="tmp8_tk"),
                        "t_q": f8p.tile([128, NQ], F8, name="tmp8_tq"),
                        "i_q": f8p.tile([128, NQ], F8, name="tmp8_iq"),
                        "i_k": f8p.tile([128, NI], F8, name="tmp8_ik")}
                v_t = pools["vpool"].tile([128, (M // 128) * 257], BF16, name="v_t")
                v_i = pools["vpool"].tile([128, (M // 128) * 257], BF16, name="v_i")
                v3_t = v_t.rearrange("p (mc w) -> p mc w", w=257)
                v3_i = v_i.rearrange("p (mc w) -> p mc w", w=257)

                xt = pools["xpool"].tile([128, 2, LT], F16, tag="x_t", name="xt")
                xi = pools["xpool"].tile([128, 2, NI], F16, tag="x_i", name="xi")

                def xdma(dst, src, ch, c0, c1):
                    nc.sync.dma_start(out=dst[:, ch, c0:c1],
                                      in_=src[ch, :, c0:c1])

                # ---- DMA priority order on the (shared) HWDGE/DMA path:
                # tiny weights first, then 512-token leading x chunks so the
                # first projection tile can start ~2us in. The remaining x
                # chunks and the V tiles are emitted as prologue units below,
                # interleaved so they never delay a k-tile DMA or gather the
                # stats pipeline is waiting on ----
                nc.sync.dma_start(out=wpack_t, in_=wpack[:, :, :, :])
                nc.sync.dma_start(out=bias_t, in_=biases[:, :])
                for ch in range(2):
                    xdma(xt, x_t, ch, 0, 512)
                for ch in range(2):
                    xdma(xi, x_i, ch, 0, 512)

                a1 = _attn_state(nc, pools, "a1", i_qT, t_kT, i_q8, t_k8,
                                 v_t, out_i)
                a2 = _attn_state(nc, pools, "a2", t_qT, i_kT, t_q8, i_k8,
                                 v_i, out_t)

                def ptile(dst, f8key, q8t, wname, bcol, x, n0):
                    return lambda: _proj_tile(nc, pools, dst,
                                              (tmp8[f8key], q8t), wT[wname],
                                              bias_t[:, bcol:bcol + 1], x, n0)

                A = [ptile(t_kT, "t_k", t_k8, "wtk", 2, xt, n0)
                     for n0 in range(0, LT, 512)]
                Bq = [ptile(i_qT, "i_q", i_q8, "wq", 0, xi, n0)
                      for n0 in range(0, NQ, 512)]
                T = [ptile(t_qT, "t_q", t_q8, "wtq", 3, xt, n0)
                     for n0 in range(0, NQ, 512)]
                K = [ptile(i_kT, "i_k", i_k8, "wk", 1, xi, n0)
                     for n0 in range(0, NI, 512)]

                # ---- prologue: ALL projection tiles + the stats pairs of
                # quarters a1-0 AND a1-1, in dependency-local order
                # (p-major stats so each fresh pair of t_k tiles unlocks
                # four more pairs; projection tiles ordered by x-chunk
                # arrival so the in-order PE queue never blocks a ready
                # stats pair behind a waiting one) ----
                cells = [(a1, q) for q in range(NQ // 512)] + \
                        [(a2, q) for q in range(NQ // 512)]
                s0 = _stats_pairs(nc, pools, a1, 0)
                xmid = lambda xv, xd: lambda: [xdma(xv, xd, ch, 512, 2048)
                                               for ch in range(2)]
                xlast = lambda xv, xd: lambda: [xdma(xv, xd, ch, 2048, 4096)
                                                for ch in range(2)]
                vload = lambda v3, vd: lambda: nc.sync.dma_start(
                    out=v3, in_=vd[:, :, :])
                # NOTE x chunk coverage: lead 0:512, mid 512:2048,
                # last 2048:4096 -- every unit reading a chunk must be
                # emitted AFTER that chunk's DMA (the dependency tracker
                # follows emission order; a read emitted first silently
                # reads stale memory)
                prologue = ([A[0], Bq[0], T[0], xmid(xt, x_t), A[1]]
                            + s0[0:4]
                            + [A[2], A[3], xmid(xi, x_i), T[1], Bq[1], K[0]]
                            + s0[4:8]
                            + [xlast(xt, x_t), A[4], A[5], T[2], T[3],
                               Bq[2], K[1]]
                            + s0[8:12]
                            + [vload(v3_t, v_td), xlast(xi, x_i),
                               A[6], A[7], Bq[3], K[2], K[3]]
                            + s0[12:16]
                            + _stats_fins(nc, pools, a1, 0)
                            + [K[4], K[5], K[6], K[7], vload(v3_i, v_id)]
                            + _stats_pairs(nc, pools, a1, 1))
                for u in prologue:
                    u()

                # ---- steady state runs stats one quarter ahead: while cell
                # i computes, emit cell i+1's finalizes (pairs reduced last
                # cell) and cell i+2's pairs ----
                for i, (att, q) in enumerate(cells):
                    fillers = []
                    if i + 1 < len(cells):
                        fillers += _stats_fins(nc, pools, *cells[i + 1])
                    if i + 2 < len(cells):
                        fillers += _stats_pairs(nc, pools, *cells[i + 2])
                    _attn_av_quarter(nc, pools, att, q, fillers)
                    _attn_store_quarter(nc, pools, att, q)

    nc.compile()
    return nc


_NC_CACHE = {}


def _get_nc():
    if "nc" not in _NC_CACHE:
        _NC_CACHE["nc"] = build_nc()
    return _NC_CACHE["nc"]


def _swapped(x_flat, h):
    """x_flat [C, Ntok] fp16, token halves swapped when h==1."""
    x_flat = x_flat.astype(np.float16)
    if h:
        n = x_flat.shape[1]
        x_flat = np.concatenate([x_flat[:, n // 2:], x_flat[:, :n // 2]], axis=1)
    return x_flat


def _v_pack(xs):
    """xs [C, M] -> [128, M/128, 257] x^T chunks + ones column (bf16)."""
    v = np.empty((128, M // 128, 257), ml_dtypes.bfloat16)
    v[:, :, 0:256] = xs.astype(np.float32).T.reshape(
        M // 128, 128, C).transpose(1, 0, 2).astype(ml_dtypes.bfloat16)
    v[:, :, 256] = 1.0
    return np.ascontiguousarray(v)


def _fold_weight(w, scale):
    """w [Hd, C] * per-row scale -> pre-transposed fp16 [c-in-half, ch, hd]."""
    wp = (w * scale[:, None]).astype(np.float32)
    arr = wp.T.reshape(2, 128, 128).transpose(1, 0, 2)
    return np.ascontiguousarray(arr.astype(np.float16))


def run_spmd(inputs, **kw):
    """Build in_maps, run on 8 cores, return BassKernelResults."""
    nc = _get_nc()
    s_q = inputs["bnq_gamma"] / np.sqrt(inputs["bnq_var"] + BN_EPS)
    s_k = inputs["bnk_gamma"] / np.sqrt(inputs["bnk_var"] + BN_EPS)
    wq = _fold_weight(inputs["w_img_q"], s_q * SQ)
    wk = _fold_weight(inputs["w_img_k"], s_k * SQ)
    wtq = _fold_weight(inputs["w_text_q"], np.full(HD, SQ, np.float32))
    wtk = _fold_weight(inputs["w_text_k"], np.full(HD, SQ, np.float32))
    wpack = np.ascontiguousarray(np.stack([wq, wk, wtq, wtk], axis=2))
    biases = np.ascontiguousarray(np.stack([
        (inputs["bnq_beta"] - inputs["bnq_mean"] * s_q) * SQ,
        (inputs["bnk_beta"] - inputs["bnk_mean"] * s_k) * SQ,
        inputs["b_text_k"] * SQ,
        inputs["b_text_q"] * SQ,
    ], axis=1).astype(np.float32))
    in_maps = []
    for core in range(N_CORES):
        b, h = core // 2, core % 2
        xsi = _swapped(inputs["input_i"][b].reshape(C, NI), h)
        xst = _swapped(inputs["input_t"][b].reshape(C, LT), h)
        m = {
            "x_i": np.ascontiguousarray(xsi.reshape(2, 128, NI)),
            "x_t": np.ascontiguousarray(xst.reshape(2, 128, LT)),
            "v_td": _v_pack(xst), "v_id": _v_pack(xsi),
            "wpack": wpack, "biases": biases,
        }
        in_maps.append(m)
    res = run_bass_kernel_spmd(nc, in_maps, list(range(N_CORES)), **kw)
    return res


def gather(res):
    output_i = np.empty((B, NI, C), np.float32)
    output_t = np.empty((B, LT, C), np.float32)
    for core in range(N_CORES):
        b, h = core // 2, core % 2
        r = res.results[core]
        output_i[b, h * NQ:(h + 1) * NQ, :] = np.asarray(r["out_i"])
        output_t[b, h * NQ:(h + 1) * NQ, :] = np.asarray(r["out_t"])
    return (output_i, output_t)


def kernel(**inputs):
    inputs = {k: np.asarray(v, dtype=np.float32) for k, v in inputs.items()}
    res = run_spmd(inputs)
    return gather(res)


# revision 29
# speedup vs baseline: 1.1435x; 1.1070x over previous
"""Cross-attention (image<->text) kernel for TRN2, 8-core SPMD.

Problem: nn_CrossAttention. B=4, C=256, H=W=64 (Ni=4096), Lt=4096, Hd=128.

Sharding: 8 cores = 4 batches x 2 query-token halves. Each core computes
  att1 (img->text): queries = image tokens [half of 4096], keys/values = all text tokens
  att2 (text->img): queries = text tokens [half of 4096], keys/values = all image tokens
Outputs are disjoint slices -> no collectives; host gathers + transposes.

Per-core algorithm (fp32 PSUM accumulation throughout):
  - Host folds BN (eval) + the sqrt(Hd) score scale into the projection
    weights, pre-transposes them, and converts x to fp16. V^T tiles (plus a
    ones column that yields the softmax denominator inside the AV
    accumulation) come host-pretransposed in bf16.
  - Projections (fp16) evict PSUM->SBUF via ACT twice: once fp16 (exact
    q/k) and once fp8e4m3 (stats copy); two SWDGE DMAs (Pool queue, off the
    shared HWDGE) lay the fp8 copy out as [64, 2, N] k-tile pairs for
    DoubleRow matmuls.
  - Pass 1 (stats) runs entirely in fp8 DoubleRow (0.5 PE cycles/row):
    S8 = Q8 K8^T in [q, m] tiles; per-row max via DVE negated max-reduce.
    The resulting per-query shift b is only a ~+-30-accurate estimate of
    the true row max -- safe because P~ is bf16 (exp range e^+-88) and any
    consistent per-row shift cancels exactly in the final normalization.
  - Pass 2: exact fp16 scores S^T in [m, q] layout; the per-query -b shift
    is added with a single fp8 DoubleRow rank-1 matmul: lhsT = a row of
    2.0s, rhs = the -b/4 row broadcast (stride-0) over both k-tiles, so
    2*(-b/4)*2 = -b. (/4 keeps magnitudes < ~170: the device float8e4 is
    IEEE e4m3, max finite 240 then inf.) Half the PE cost of an fp16
    rank-1 pair. exp on ACT -> bf16 P~; AV: out[q, C+1] += P~^T.T @ [V|1];
    normalization is one batched reciprocal + per-row scale at eviction.
  - Scheduling: DMA priority order (tiny weights first, 512-token leading
    x chunks) so the first projection starts ~2us in; stats pairs are
    emitted as soon as their q8/k8 chunks exist (p-major); the AV mc loop
    is software-pipelined (next chunk's score matmuls precede this chunk's
    AV) with next-quarter stats pairs + remaining projection tiles
    distributed over the mc slots; unnormalized-output staging runs on the
    otherwise-idle Pool engine so the next quarter's first AV matmul isn't
    WAR-blocked behind ACT's exp queue.
"""

import numpy as np
import ml_dtypes

import concourse.bacc as bacc
import concourse.tile as tile
from concourse import mybir
from concourse.bass_utils import run_bass_kernel_spmd

F32 = mybir.dt.float32
F16 = mybir.dt.float16
BF16 = mybir.dt.bfloat16
F8 = mybir.dt.float8e4
AF = mybir.ActivationFunctionType
ALU = mybir.AluOpType
AX = mybir.AxisListType
DRM = mybir.MatmulPerfMode.DoubleRow

B, C, HD = 4, 256, 128
NI, LT = 4096, 4096
NQ = 2048        # query tokens per core (half)
M = 4096         # kv tokens (full)
SQ = float(128.0 ** 0.25)   # sqrt(sqrt(Hd)) folded into each of q and k
BN_EPS = 1e-5

N_CORES = 8


def _proj_tile(nc, pools, out_t, f8, wT, bias_ap, x, n0):
    """One 512-token projection tile: 2 matmuls (c halves) + fp16 ACT
    eviction + fp8 ACT eviction into tmp8, + 2 SWDGE DMAs laying the fp8
    copy out as [64, 2, 512] k-tile pairs (hd = t*64 + p) for DoubleRow.
    Uses the ps_p2 ring (keeps the stats pairs' ps_p1 ring free)."""
    ps = pools["ps_p2"].tile([128, 512], F32, tag="ps_p2", name="ps_prj")
    for ch in range(2):
        nc.tensor.matmul(ps, wT[:, ch, :], x[:, ch, n0:n0 + 512],
                         start=(ch == 0), stop=(ch == 1))
    nc.scalar.activation(out_t[:, n0:n0 + 512], ps, AF.Identity,
                         bias=bias_ap, scale=1.0)
    tmp8, qk8 = f8
    nc.scalar.activation(tmp8[:, n0:n0 + 512], ps, AF.Identity,
                         bias=bias_ap, scale=1.0)
    # HWDGE (sync), not SWDGE: the Pool desc-gen path serializes at ~1us
    # per DMA and a 48-DMA prologue backlog there starves the stats pairs
    nc.sync.dma_start(out=qk8[:, 0, n0:n0 + 512], in_=tmp8[0:64, n0:n0 + 512])
    nc.sync.dma_start(out=qk8[:, 1, n0:n0 + 512],
                      in_=tmp8[64:128, n0:n0 + 512])


def _stats_pair(nc, pools, st, qt, p):
    """One pass-1 unit: 2 fp8 DoubleRow score matmuls into one 2-bank psum
    tile + ONE negated max-reduce [128,2,512]->[128,2] on DVE."""
    q8, k8 = st["q8"], st["k8"]
    q_sl = q8[:, :, qt * 128:(qt + 1) * 128]
    ps = pools["ps_p1"].tile([128, 1024], F32, tag="ps_p1", name="ps_s",
                             bufs=1)
    for j in range(2):
        w = 2 * p + j
        nc.tensor.matmul(ps[:, j * 512:(j + 1) * 512], q_sl,
                         k8[:, :, w * 512:(w + 1) * 512],
                         start=True, stop=True, perf_mode=DRM)
    nc.vector.tensor_reduce(st["negm_all"][:, qt, 2 * p:2 * p + 2],
                            ps.rearrange("a (j w) -> a j w", w=512),
                            axis=AX.X, op=ALU.max, negate=True)


def _attn_stats_finalize_half(nc, pools, st, nq, h):
    """Combine pair maxes for qtile pair h of quarter nq -> fp8 -b/4 values,
    scattered into the quarter's bias row in linear-q order (2 gathers)."""
    sp, tag = pools["stats"], st["tag"]
    sl = slice(nq * 4 + 2 * h, nq * 4 + 2 * h + 2)
    neg_mt = sp.tile([128, 2], F32, name=f"negM_{tag}_{nq}_{h}",
                     tag=f"negM_{tag}", bufs=2)
    nc.vector.tensor_reduce(neg_mt, st["negm_all"][:, sl, :], axis=AX.X,
                            op=ALU.min)
    neg_b8 = sp.tile([128, 2], F8, tag=f"negb8_{tag}",
                     name=f"negb8_{tag}_{nq}_{h}", bufs=2)
    nc.vector.tensor_scalar_mul(neg_b8, neg_mt, 0.25)
    # store the -b/4 values physically in BOTH k-tiles (no stride-0
    # broadcast AP on the matmul read: its writes-dependency tracking is
    # what orders the bias matmul after these gathers). HWDGE (sync), not
    # SWDGE: these gathers gate the next quarter's bias matmuls, and the
    # Pool desc-gen path adds ~1us each.
    for t in range(2):
        for s in range(2):
            b0 = nq * 512 + h * 256 + s * 128
            nc.sync.dma_start(out=st["negb8"][0:1, t, b0:b0 + 128],
                              in_=neg_b8[:, s:s + 1])


def _stats_pairs(nc, pools, st, nq):
    """16 pair closures for one quarter's stats, p-major (each fresh pair of
    k8 tiles unlocks four more pairs)."""
    return [lambda qt=nq * 4 + i, p=p: _stats_pair(nc, pools, st, qt, p)
            for p in range(4) for i in range(4)]


def _stats_fins(nc, pools, st, nq):
    return [lambda h=h: _attn_stats_finalize_half(nc, pools, st, nq, h)
            for h in range(2)]


def _attn_av_quarter(nc, pools, st, nq, fillers=()):
    """Pass-2 S^T + DoubleRow rank-1 bias + exp + AV for one 512-query
    quarter.

    mc loop software-pipelined by one iteration (bias+score matmuls of chunk
    mc+1 are emitted BEFORE the AV matmuls of chunk mc); `fillers`
    (next-quarter stats + remaining projection tiles) are spread evenly
    over the 32 mc slots."""
    qT, kT, v, tag = st["qT"], st["kT"], st["v"], st["tag"]
    twos8 = pools["twos8"]
    n_mc = M // 128
    v3 = v.rearrange("p (mc w) -> p mc w", w=257)
    n0 = nq * 512
    negb8_b = st["negb8"][0:1, :, n0:n0 + 512]
    out_ps = [pools["ps_out"].tile([128, 257], F32, tag=f"out_ps{s}",
                                   name=f"out_ps{s}_{tag}_{nq}")
              for s in range(4)]
    st["out_ps"] = out_ps

    def scores(mc):
        ps = pools["ps_p2"].tile([128, 512], F32, tag="ps_p2", name="ps_p2c")
        # the full-width score matmul OPENS the group (it depends only on
        # qT/kT); the DoubleRow bias closes it, so a late-arriving finalize
        # DMA never blocks the score matmul itself
        nc.tensor.matmul(ps, kT[:, mc * 128:(mc + 1) * 128],
                         qT[:, n0:n0 + 512], start=True, stop=False)
        nc.tensor.matmul(ps, twos8, negb8_b, start=False, stop=True,
                         perf_mode=DRM, skip_group_check=True)
        pt = pools["pt"].tile([128, 512], BF16, tag="pt",
                              name=f"pt_{tag}_{nq}_{mc}")
        nc.scalar.activation(pt, ps, AF.Exp)
        return pt

    fillers = list(fillers)
    # stats run one quarter ahead: fillers = [fin(q+1) x2, pairs(q+2) x16].
    # The two fins go in the first two slots (their inputs were reduced last
    # quarter; the bias-row gathers then land ~30 mc before they're needed);
    # pairs are spaced 2 mc apart so a pair's matmuls never WAR-stall on the
    # single-buffer ps_p1 tile still being reduced.
    slot = {}
    for k in range(len(fillers)):
        slot.setdefault(k if k < 2 else min(2 * k - 2, n_mc - 1), []).append(k)
    # software-pipeline the scores TWO chunks ahead: AV(mc) then never waits
    # on exp(mc) (which completes during AV(mc-1)), and the ps_p2 buf that
    # scores(mc+2) reuses is freed by exp(mc) just in time
    pts = [scores(0), scores(1)]
    for mc in range(n_mc):
        if mc + 2 < n_mc:
            pts.append(scores(mc + 2))
        pt_cur = pts[mc]
        for s in range(4):
            nc.tensor.matmul(out_ps[s], pt_cur[:, s * 128:(s + 1) * 128],
                             v3[:, mc, :],
                             start=(mc == 0), stop=(mc == n_mc - 1))
        for k in slot.get(mc, ()):
            fillers[k]()
        pts[mc] = None


def _attn_store_quarter(nc, pools, st, nq):
    """Normalize straight out of PSUM on DVE (per-s reciprocal + scale) +
    store DMA. Skipping a raw-staging copy frees each out_ps tile ~0.5us
    after its last AV matmul, so the next quarter's first AV matmul isn't
    WAR-blocked behind an eviction queue."""
    sp, tag, out_dram = pools["stats"], st["tag"], st["out_dram"]
    out_ps = st["out_ps"]
    for s in range(4):
        qt = nq * 4 + s
        recip = sp.tile([128, 1], F32, tag=f"recip_{tag}",
                        name=f"recip_{tag}_{qt}", bufs=2)
        nc.vector.reciprocal(recip, out_ps[s][:, 256:257])
        ostage = pools["ostage"].tile([128, 256], F32, tag="ostage",
                                      name=f"ostage_{tag}_{qt}")
        nc.vector.tensor_scalar_mul(ostage, out_ps[s][:, 0:256], recip)
        nc.sync.dma_start(out=out_dram[qt * 128:(qt + 1) * 128, :], in_=ostage)


def _attn_state(nc, pools, tag, qT, kT, q8, k8, v, out_dram):
    sp = pools["stats"]
    n_qt = NQ // 128
    st = {"tag": tag, "qT": qT, "kT": kT, "q8": q8, "k8": k8, "v": v,
          "out_dram": out_dram}
    st["negb8"] = sp.tile([1, 2, NQ], F8, name=f"negb8row_{tag}",
                          tag=f"negb8row_{tag}", bufs=1)
    st["negm_all"] = sp.tile([128, n_qt, 8], F32, name=f"negmall_{tag}",
                             tag=f"negmall_{tag}", bufs=1)
    return st


def build_nc(reps=1):
    """Build the SPMD Bass program (identical for all cores).

    Query tokens are always slice [0:NQ] of the token axis. The host feeds
    half-1 cores an x whose two token halves are swapped: queries then sit in
    [0:NQ], while keys/values see a permuted-but-consistent full token set
    (softmax+AV are invariant to a joint permutation of keys and values)."""
    nc = bacc.Bacc(None)

    x_i = nc.dram_tensor("x_i", [2, 128, NI], F16, kind="ExternalInput")
    x_t = nc.dram_tensor("x_t", [2, 128, LT], F16, kind="ExternalInput")
    # host-pretransposed V (+ ones column): [m-in-chunk, mc, C+1]
    v_td = nc.dram_tensor("v_td", [128, M // 128, 257], BF16, kind="ExternalInput")
    v_id = nc.dram_tensor("v_id", [128, M // 128, 257], BF16, kind="ExternalInput")
    # host-folded, pre-transposed fp16 weights, packed into ONE tensor
    # [c-in-half, ch, which(wq,wk,wtq,wtk), hd] so the prologue needs a
    # single tiny weight DMA before the first projection can start
    wpack = nc.dram_tensor("wpack", [128, 2, 4, 128], F16, kind="ExternalInput")
    # host-folded projection biases [128, 4]: i_q, i_k, t_k, t_q
    biases = nc.dram_tensor("biases", [128, 4], F32, kind="ExternalInput")
    out_i = nc.dram_tensor("out_i", [NQ, C], F32, kind="ExternalOutput")
    out_t = nc.dram_tensor("out_t", [NQ, C], F32, kind="ExternalOutput")

    with tile.TileContext(nc) as tc:
        import contextlib
        with contextlib.ExitStack() as ctx:

            pools = {}
            pools["consts"] = ctx.enter_context(tc.tile_pool(name="consts", bufs=1))
            pools["stats"] = ctx.enter_context(tc.tile_pool(name="stats", bufs=2))
            pools["xpool"] = ctx.enter_context(tc.tile_pool(name="xpool", bufs=1))
            pools["vpool"] = ctx.enter_context(tc.tile_pool(name="vpool", bufs=1))
            pools["qkpool"] = ctx.enter_context(tc.tile_pool(name="qkpool", bufs=1))
            pools["f8pool"] = ctx.enter_context(tc.tile_pool(name="f8pool", bufs=1))
            pools["pt"] = ctx.enter_context(tc.tile_pool(name="pt", bufs=5))
            pools["ostage"] = ctx.enter_context(tc.tile_pool(name="ostage", bufs=4))
            pools["ps_p1"] = ctx.enter_context(
                tc.tile_pool(name="ps_p1", bufs=2, space="PSUM"))
            pools["ps_p2"] = ctx.enter_context(
                tc.tile_pool(name="ps_p2", bufs=2, space="PSUM"))
            pools["ps_out"] = ctx.enter_context(
                tc.tile_pool(name="ps_out", bufs=1, space="PSUM"))

            consts = pools["consts"]

            wpack_t = consts.tile([128, 2, 4, 128], F16, name="wpack_t")
            wT = {name: wpack_t[:, :, i, :]
                  for i, name in enumerate(("wq", "wk", "wtq", "wtk"))}
            bias_t = consts.tile([128, 4], F32, name="bias_t")

            twos_f = consts.tile([1, 256], F32, name="twos_row_f")
            nc.vector.memset(twos_f, 2.0)
            twos8_t = consts.tile([1, 2, 128], F8, name="twos8_row")
            nc.vector.tensor_copy(twos8_t.rearrange("o t p -> o (t p)"), twos_f)
            pools["twos8"] = twos8_t

            for _rep in range(reps):
                qk = pools["qkpool"]
                t_kT = qk.tile([128, LT], F16, name="t_kT")
                t_qT = qk.tile([128, NQ], F16, name="t_qT")
                i_qT = qk.tile([128, NQ], F16, name="i_qT")
                i_kT = qk.tile([128, NI], F16, name="i_kT")
                f8p = pools["f8pool"]
                t_k8 = f8p.tile([64, 2, LT], F8, name="t_k8")
                t_q8 = f8p.tile([64, 2, NQ], F8, name="t_q8")
                i_q8 = f8p.tile([64, 2, NQ], F8, name="i_q8")
                i_k8 = f8p.tile([64, 2, NI], F8, name="i_k8")
                tmp8 = {"t_k": f8p.tile([128, LT], F8, name="tmp8_tk"),
                        "t_q": f8p.tile([128, NQ], F8, name="tmp8_tq"),
                        "i_q": f8p.tile([128, NQ], F8, name="tmp8_iq"),
                        "i_k": f8p.tile([128, NI], F8, name="tmp8_ik")}
                v_t = pools["vpool"].tile([128, (M // 128) * 257], BF16, name="v_t")
                v_i = pools["vpool"].tile([128, (M // 128) * 257], BF16, name="v_i")
                v3_t = v_t.rearrange("p (mc w) -> p mc w", w=257)
                v3_i = v_i.rearrange("p (mc w) -> p mc w", w=257)

                xt = pools["xpool"].tile([128, 2, LT], F16, tag="x_t", name="xt")
                xi = pools["xpool"].tile([128, 2, NI], F16, tag="x_i", name="xi")

                def xdma(dst, src, ch, c0, c1):
                    nc.sync.dma_start(out=dst[:, ch, c0:c1],
                                      in_=src[ch, :, c0:c1])

                # ---- DMA priority order on the (shared) HWDGE/DMA path:
                # tiny weights first, then 512-token leading x chunks so the
                # first projection tile can start ~2us in. The remaining x
                # chunks and the V tiles are emitted as prologue units below,
                # interleaved so they never delay a k-tile DMA or gather the
                # stats pipeline is waiting on ----
                nc.sync.dma_start(out=wpack_t, in_=wpack[:, :, :, :])
                nc.sync.dma_start(out=bias_t, in_=biases[:, :])
                for ch in range(2):
                    xdma(xt, x_t, ch, 0, 512)
                for ch in range(2):
                    xdma(xi, x_i, ch, 0, 512)

                a1 = _attn_state(nc, pools, "a1", i_qT, t_kT, i_q8, t_k8,
                                 v_t, out_i)
                a2 = _attn_state(nc, pools, "a2", t_qT, i_kT, t_q8, i_k8,
                                 v_i, out_t)

                def ptile(dst, f8key, q8t, wname, bcol, x, n0):
                    return lambda: _proj_tile(nc, pools, dst,
                                              (tmp8[f8key], q8t), wT[wname],
                                              bias_t[:, bcol:bcol + 1], x, n0)

                A = [ptile(t_kT, "t_k", t_k8, "wtk", 2, xt, n0)
                     for n0 in range(0, LT, 512)]
                Bq = [ptile(i_qT, "i_q", i_q8, "wq", 0, xi, n0)
                      for n0 in range(0, NQ, 512)]
                T = [ptile(t_qT, "t_q", t_q8, "wtq", 3, xt, n0)
                     for n0 in range(0, NQ, 512)]
                K = [ptile(i_kT, "i_k", i_k8, "wk", 1, xi, n0)
                     for n0 in range(0, NI, 512)]

                # ---- prologue: ALL projection tiles + the stats pairs of
                # quarters a1-0 AND a1-1, in dependency-local order
                # (p-major stats so each fresh pair of t_k tiles unlocks
                # four more pairs; projection tiles ordered by x-chunk
                # arrival so the in-order PE queue never blocks a ready
                # stats pair behind a waiting one) ----
                cells = [(a1, q) for q in range(NQ // 512)] + \
                        [(a2, q) for q in range(NQ // 512)]
                s0 = _stats_pairs(nc, pools, a1, 0)
                xmid = lambda xv, xd: lambda: [xdma(xv, xd, ch, 512, 2048)
                                               for ch in range(2)]
                xlast = lambda xv, xd: lambda: [xdma(xv, xd, ch, 2048, 4096)
                                                for ch in range(2)]
                vload = lambda v3, vd: lambda: nc.sync.dma_start(
                    out=v3, in_=vd[:, :, :])
                # NOTE x chunk coverage: lead 0:512, mid 512:2048,
                # last 2048:4096 -- every unit reading a chunk must be
                # emitted AFTER that chunk's DMA (the dependency tracker
                # follows emission order; a read emitted first silently
                # reads stale memory)
                prologue = ([A[0], Bq[0], T[0], xmid(xt, x_t), A[1]]
                            + s0[0:4]
                            + [A[2], A[3], xmid(xi, x_i), T[1], Bq[1], K[0]]
                            + s0[4:8]
                            + [xlast(xt, x_t), A[4], A[5], T[2], T[3],
                               Bq[2], K[1]]
                            + s0[8:12]
                            + [vload(v3_t, v_td), xlast(xi, x_i),
                               A[6], A[7], Bq[3], K[2], K[3]]
                            + s0[12:16]
                            + _stats_fins(nc, pools, a1, 0)
                            + [K[4], K[5], K[6], K[7], vload(v3_i, v_id)]
                            + _stats_pairs(nc, pools, a1, 1))
                for u in prologue:
                    u()

                # ---- steady state runs stats one quarter ahead: while cell
                # i computes, emit cell i+1's finalizes (pairs reduced last
                # cell) and cell i+2's pairs ----
                for i, (att, q) in enumerate(cells):
                    fillers = []
                    if i + 1 < len(cells):
                        fillers += _stats_fins(nc, pools, *cells[i + 1])
                    if i + 2 < len(cells):
                        fillers += _stats_pairs(nc, pools, *cells[i + 2])
                    _attn_av_quarter(nc, pools, att, q, fillers)
                    _attn_store_quarter(nc, pools, att, q)

    nc.compile()
    return nc


_NC_CACHE = {}


def _get_nc():
    if "nc" not in _NC_CACHE:
        _NC_CACHE["nc"] = build_nc()
    return _NC_CACHE["nc"]


def _swapped(x_flat, h):
    """x_flat [C, Ntok] fp16, token halves swapped when h==1."""
    x_flat = x_flat.astype(np.float16)
    if h:
        n = x_flat.shape[1]
        x_flat = np.concatenate([x_flat[:, n // 2:], x_flat[:, :n // 2]], axis=1)
    return x_flat


def _v_pack(xs):
    """xs [C, M] -> [128, M/128, 257] x^T chunks + ones column (bf16)."""
    v = np.empty((128, M // 128, 257), ml_dtypes.bfloat16)
    v[:, :, 0:256] = xs.astype(np.float32).T.reshape(
        M // 128, 128, C).transpose(1, 0, 2).astype(ml_dtypes.bfloat16)
    v[:, :, 256] = 1.0
    return np.ascontiguousarray(v)


def _fold_weight(w, scale):
    """w [Hd, C] * per-row scale -> pre-transposed fp16 [c-in-half, ch, hd]."""
    wp = (w * scale[:, None]).astype(np.float32)
    arr = wp.T.reshape(2, 128, 128).transpose(1, 0, 2)
    return np.ascontiguousarray(arr.astype(np.float16))


def run_spmd(inputs, **kw):
    """Build in_maps, run on 8 cores, return BassKernelResults."""
    nc = _get_nc()
    s_q = inputs["bnq_gamma"] / np.sqrt(inputs["bnq_var"] + BN_EPS)
    s_k = inputs["bnk_gamma"] / np.sqrt(inputs["bnk_var"] + BN_EPS)
    wq = _fold_weight(inputs["w_img_q"], s_q * SQ)
    wk = _fold_weight(inputs["w_img_k"], s_k * SQ)
    wtq = _fold_weight(inputs["w_text_q"], np.full(HD, SQ, np.float32))
    wtk = _fold_weight(inputs["w_text_k"], np.full(HD, SQ, np.float32))
    wpack = np.ascontiguousarray(np.stack([wq, wk, wtq, wtk], axis=2))
    biases = np.ascontiguousarray(np.stack([
        (inputs["bnq_beta"] - inputs["bnq_mean"] * s_q) * SQ,
        (inputs["bnk_beta"] - inputs["bnk_mean"] * s_k) * SQ,
        inputs["b_text_k"] * SQ,
        inputs["b_text_q"] * SQ,
    ], axis=1).astype(np.float32))
    in_maps = []
    for core in range(N_CORES):
        b, h = core // 2, core % 2
        xsi = _swapped(inputs["input_i"][b].reshape(C, NI), h)
        xst = _swapped(inputs["input_t"][b].reshape(C, LT), h)
        m = {
            "x_i": np.ascontiguousarray(xsi.reshape(2, 128, NI)),
            "x_t": np.ascontiguousarray(xst.reshape(2, 128, LT)),
            "v_td": _v_pack(xst), "v_id": _v_pack(xsi),
            "wpack": wpack, "biases": biases,
        }
        in_maps.append(m)
    res = run_bass_kernel_spmd(nc, in_maps, list(range(N_CORES)), **kw)
    return res


def gather(res):
    output_i = np.empty((B, NI, C), np.float32)
    output_t = np.empty((B, LT, C), np.float32)
    for core in range(N_CORES):
        b, h = core // 2, core % 2
        r = res.results[core]
        output_i[b, h * NQ:(h + 1) * NQ, :] = np.asarray(r["out_i"])
        output_t[b, h * NQ:(h + 1) * NQ, :] = np.asarray(r["out_t"])
    return (output_i, output_t)


def kernel(**inputs):
    inputs = {k: np.asarray(v, dtype=np.float32) for k, v in inputs.items()}
    res = run_spmd(inputs)
    return gather(res)
